# revision 13
# baseline (speedup 1.0000x reference)
"""MFN (Memory Fusion Network) Trainium2 Bass kernel.

Data-parallel over batch n=4096 across 8 NeuronCores (512/core).
Feature-major layout on chip: [feature_partitions, batch_free].
All matmuls bf16 (fp32 PSUM accumulation). Sigmoid via 0.5*tanh(x/2)+0.5
so a single ACT table set (exp/tanh/relu/identity) covers every step.
"""
import os
import sys

sys.path.insert(0, "/opt/trn_rl_repo")

import numpy as np
import ml_dtypes
from contextlib import ExitStack

import concourse.bass as bass
import concourse.tile as tile
from concourse import mybir, bacc
from concourse.bass_utils import run_bass_kernel_spmd

# ---------------- problem constants (hardcoded) ----------------
T = int(os.environ.get("MFN_T", "64"))
NFULL = 4096
NCORES = 8
B = 512                      # batch per core
D_EMB, D_A, D_V = 300, 128, 128
EP = 384                     # padded embedding row (384 bf16 = 768B, mult of 256)
DH_L, DH_A, DH_V = 256, 128, 128
MEM = 512
HID = 512
VOCAB = 32000

BF16 = mybir.dt.bfloat16
F32 = mybir.dt.float32
AF = mybir.ActivationFunctionType

# LSTM gate row permutation: [i, f, o, g] (natural order is i, f, g, o)
def _gate_perm(dh):
    idx = np.arange(4 * dh)
    return np.concatenate([idx[0:dh], idx[dh:2*dh], idx[3*dh:4*dh], idx[2*dh:3*dh]])


# ---------------- weight blob layout ----------------
# Walked identically by host packer and bass builder.
# Entries: (tag, n_m, n_k, width_per_tile)
_LAYERS = [
    ("z_l", 8, 5, 128),      # per m: 3 x-tiles (Wih_l, K padded to 384) + 2 h-tiles (Whh_l)
    ("z_a", 4, 2, 128),      # per m: 1 x + 1 h
    ("z_v", 4, 2, 128),
    ("att1_l1", 4, 8, 128),
    ("att1_l2", 8, 4, 128),
    ("att2_l1", 4, 8, 128),
    ("att2_l2", 4, 4, 128),
    ("g1_l1", 4, 12, 128),
    ("g1_l2", 4, 4, 128),
    ("g2_l1", 4, 12, 128),
    ("g2_l2", 4, 4, 128),
    ("out_l1", 4, 8, 128),
    ("out_l2", 1, 4, 1),     # M=1 tiles
]

def _blob_offsets():
    offs = {}
    cur = 0
    for tag, n_m, n_k, w in _LAYERS:
        offs[tag] = []
        for m in range(n_m):
            row = []
            for k in range(n_k):
                row.append((cur, w))
                cur += w
            offs[tag].append(row)
    return offs, cur

_W_OFFS, _WCOLS = _blob_offsets()

# bias blob columns, in ACT-call order
_B_ORDER = [
    ("zl_if", 4), ("zl_o", 2), ("zl_g", 2),
    ("za_ifo", 3), ("za_g", 1),
    ("zv_ifo", 3), ("zv_g", 1),
    ("att1_h", 4), ("att1_z", 8),
    ("att2_h", 4), ("att2_o", 4),
    ("g1_h", 4), ("g1_o", 4),
    ("g2_h", 4), ("g2_o", 4),
    ("out_h", 4), ("out_o", 1),
]
_B_OFFS = {}
_c = 0
for _name, _n in _B_ORDER:
    _B_OFFS[_name] = _c
    _c += _n
_BCOLS = _c


# ---------------- host packing ----------------
def _lhsT_tiles(w, n_k, n_m, k_pad=None):
    """w: [dout, din] -> list over (m, k) of [128, 128] lhsT tiles (bf16)."""
    dout, din = w.shape
    kp = k_pad if k_pad is not None else din
    wt = np.zeros((kp, dout), np.float32)
    wt[:din, :] = w.T
    tiles = []
    for m in range(n_m):
        for k in range(n_k):
            tiles.append(wt[128*k:128*(k+1), 128*m:128*(m+1)])
    return tiles


def _pack_weights(p):
    """Build W_blob [128, _WCOLS] bf16 from params dict (numpy fp32)."""
    blob = np.zeros((128, _WCOLS), np.float32)

    def put(tag, m, k, arr):
        off, w = _W_OFFS[tag][m][k]
        blob[:arr.shape[0], off:off+arr.shape[1]] = arr

    # LSTMs: permute gate rows to [i,f,o,g]
    for nm, dh, din in (("l", DH_L, D_EMB), ("a", DH_A, D_A), ("v", DH_V, D_V)):
        perm = _gate_perm(dh)
        wih = np.asarray(p[f"Wih_{nm}"])[perm]          # [4dh, din]
        whh = np.asarray(p[f"Whh_{nm}"])[perm]          # [4dh, dh]
        n_m = (4 * dh) // 128
        if nm == "l":
            xt = _lhsT_tiles(wih, 3, n_m, k_pad=EP)     # K padded to 384
            ht = _lhsT_tiles(whh, 2, n_m)
            for m in range(n_m):
                for k in range(3):
                    put("z_l", m, k, xt[m*3+k])
                for k in range(2):
                    put("z_l", m, 3+k, ht[m*2+k])
        else:
            xt = _lhsT_tiles(wih, 1, n_m)
            ht = _lhsT_tiles(whh, 1, n_m)
            for m in range(n_m):
                put(f"z_{nm}", m, 0, xt[m])
                put(f"z_{nm}", m, 1, ht[m])

    for tag, wname, n_k, n_m in (
        ("att1_l1", "att1_w1", 8, 4), ("att1_l2", "att1_w2", 4, 8),
        ("att2_l1", "att2_w1", 8, 4), ("att2_l2", "att2_w2", 4, 4),
        ("g1_l1", "g1_w1", 12, 4), ("g1_l2", "g1_w2", 4, 4),
        ("g2_l1", "g2_w1", 12, 4), ("g2_l2", "g2_w2", 4, 4),
        ("out_l1", "out_w1", 8, 4),
    ):
        tl = _lhsT_tiles(np.asarray(p[wname]), n_k, n_m)
        for m in range(n_m):
            for k in range(n_k):
                put(tag, m, k, tl[m*n_k+k])

    # out_l2: [1, 512] -> 4 tiles [128, 1]
    w2 = np.asarray(p["out_w2"]).T                       # [512, 1]
    for k in range(4):
        put("out_l2", 0, k, w2[128*k:128*(k+1), :])

    return blob.astype(ml_dtypes.bfloat16)


def _pack_biases(p):
    bb = np.zeros((128, _BCOLS), np.float32)

    def put(name, j, seg):
        bb[:, _B_OFFS[name] + j] = seg

    for nm, dh in (("l", DH_L), ("a", DH_A), ("v", DH_V)):
        perm = _gate_perm(dh)
        bs = (np.asarray(p[f"bih_{nm}"]) + np.asarray(p[f"bhh_{nm}"]))[perm]
        if nm == "l":
            for j in range(4):                            # i0,i1,f0,f1 (sig: 0.5*b)
                put("zl_if", j, 0.5 * bs[128*j:128*(j+1)])
            for j in range(2):                            # o0,o1
                put("zl_o", j, 0.5 * bs[128*(4+j):128*(5+j)])
            for j in range(2):                            # g0,g1 (tanh: b)
                put("zl_g", j, bs[128*(6+j):128*(7+j)])
        else:
            for j in range(3):                            # i,f,o
                put(f"z{nm}_ifo", j, 0.5 * bs[128*j:128*(j+1)])
            put(f"z{nm}_g", 0, bs[384:512])
    for name, bname, scale, n in (
        ("att1_h", "att1_b1", 1.0, 4), ("att1_z", "att1_b2", 1.0, 8),
        ("att2_h", "att2_b1", 1.0, 4), ("att2_o", "att2_b2", 1.0, 4),
        ("g1_h", "g1_b1", 1.0, 4), ("g1_o", "g1_b2", 0.5, 4),
        ("g2_h", "g2_b1", 1.0, 4), ("g2_o", "g2_b2", 0.5, 4),
        ("out_h", "out_b1", 1.0, 4),
    ):
        b = np.asarray(p[bname]) * scale
        for j in range(n):
            put(name, j, b[128*j:128*(j+1)])
    bb[0, _B_OFFS["out_o"]] = float(np.asarray(p["out_b2"])[0])
    return bb


def _feat_major(x, n0):
    """x [N, D] -> [128, D//128, B] (feature-major per-core tiles)."""
    d = x.shape[1]
    xt = np.ascontiguousarray(x[n0:n0+B].T)               # [D, B]
    return np.ascontiguousarray(
        xt.reshape(d // 128, 128, B).transpose(1, 0, 2))  # [128, d/128, B]


# ---------------- bass builder ----------------
_NC_CACHE = {}

def _build():
    key = T
    if key in _NC_CACHE:
        return _NC_CACHE[key]
    nc = bacc.Bacc("TRN2")
    d_wb = nc.dram_tensor("wb", [128, _WCOLS], BF16, kind="ExternalInput").ap()
    d_bb = nc.dram_tensor("bb", [128, _BCOLS], F32, kind="ExternalInput").ap()
    d_emb = nc.dram_tensor("emb", [VOCAB, EP], BF16, kind="ExternalInput").ap()
    d_idx = nc.dram_tensor("idx", [128, T * 32], mybir.dt.int16, kind="ExternalInput").ap()
    d_xav = nc.dram_tensor("xav", [T, 128, 2, B], BF16, kind="ExternalInput").ap()
    d_c0 = nc.dram_tensor("c0", [128, 4, B], F32, kind="ExternalInput").ap()
    d_m0 = nc.dram_tensor("m0", [128, 4, B], F32, kind="ExternalInput").ap()
    d_y = nc.dram_tensor("y", [1, B], F32, kind="ExternalOutput").ap()

    with tile.TileContext(nc) as tc:
        with ExitStack() as ctx:
            P = lambda name, bufs: ctx.enter_context(tc.tile_pool(name=name, bufs=bufs))
            const = P("const", 1)
            xgp = P("xg", 2)
            xap = P("xa", 2)
            cp = P("c", 2)
            cbp = P("cb", 2)
            hp = P("h", 2)
            mp = P("mem", 2)
            mbp = P("mb", 2)
            sigp = P("sig", 1)
            tgp = P("tg", 1)
            thp = P("th", 1)
            h1p = P("h1", 2)
            ep = P("e", 1)
            chp = P("chat", 1)
            gp = P("g", 1)
            tmpp = P("tmp", 2)
            smp = P("sm", 1)
            pb = ctx.enter_context(tc.tile_pool(name="pb", bufs=2, space="PSUM"))

            WMAIN = _W_OFFS["out_l1"][0][0][0]
            wb = const.tile([128, WMAIN], BF16)
            nc.gpsimd.dma_start(wb, d_wb[:, 0:WMAIN])
            wo2 = const.tile([128, 4], BF16)
            _o2 = _W_OFFS["out_l2"][0][0][0]
            nc.gpsimd.dma_start(wo2, d_wb[:, _o2:_o2+4])
            bb = const.tile([128, _BCOLS], F32)
            nc.gpsimd.dma_start(bb, d_bb)
            idxs = const.tile([128, T * 32], mybir.dt.int16)
            nc.gpsimd.dma_start(idxs, d_idx)
            ones_c = const.tile([128, 1], BF16)
            nc.vector.memset(ones_c, 1.0)
            ones_rf = const.tile([1, 128], F32)
            nc.vector.memset(ones_rf, 1.0)
            ones_r = const.tile([1, 128], mybir.dt.float32r)
            nc.vector.tensor_copy(ones_r, ones_rf)

            def W(tag, m, k):
                off, w = _W_OFFS[tag][m][k]
                return wb[0:128, off:off+w]

            def WO1(m, k):
                return wo1[0:128, (m*8+k)*128:(m*8+k)*128+128]

            def BIAS(name, j):
                return bb[:, _B_OFFS[name]+j:_B_OFFS[name]+j+1]

            c_prev = cp.tile([128, 4, B], F32, tag="c")
            nc.gpsimd.dma_start(c_prev, d_c0)
            cb_prev = cbp.tile([128, 4, B], BF16, tag="cb")
            nc.vector.tensor_copy(cb_prev, c_prev)
            mem_cur = mp.tile([128, 4, B], F32, tag="mem")
            nc.gpsimd.dma_start(mem_cur, d_m0)
            mb_cur = mbp.tile([128, 4, B], BF16, tag="mb")
            nc.vector.tensor_copy(mb_cur, mem_cur)
            h_cur = hp.tile([128, 4, B], BF16, tag="h")
            nc.vector.memset(h_cur, 0.0)

            for t in range(T):
                xg = xgp.tile([128, 3, B], BF16)
                nc.gpsimd.dma_gather(xg[:], d_emb, idxs[:, t*32:(t+1)*32],
                                     B, B, EP, transpose=True)
                xa = xap.tile([128, 2, B], BF16)
                nc.gpsimd.dma_start(xa, d_xav[t])

                sig = sigp.tile([128, 12, B], BF16)
                tg = tgp.tile([128, 4, B], BF16)

                # ---- z_l: psum A = (i0,i1,f0,f1), B = (o0,o1,g0,g1) ----
                for half in range(2):
                    pz = pb.tile([128, 4, B], F32, tag="pz")
                    for mi in range(4):
                        m = half * 4 + mi
                        for k in range(3):
                            nc.tensor.matmul(pz[:, mi, :], W("z_l", m, k), xg[:, k, :],
                                             start=(k == 0), stop=False)
                        for k in range(2):
                            nc.tensor.matmul(pz[:, mi, :], W("z_l", m, 3+k),
                                             h_cur[:, k, :], start=False, stop=(k == 1))
                    if half == 0:
                        for mi in range(4):
                            nc.scalar.activation(sig[:, mi, :], pz[:, mi, :], AF.Tanh,
                                                 bias=BIAS("zl_if", mi), scale=0.5)
                    else:
                        for mi in range(2):
                            nc.scalar.activation(sig[:, 4+mi, :], pz[:, mi, :], AF.Tanh,
                                                 bias=BIAS("zl_o", mi), scale=0.5)
                        for mi in range(2):
                            nc.scalar.activation(tg[:, mi, :], pz[:, 2+mi, :], AF.Tanh,
                                                 bias=BIAS("zl_g", mi), scale=1.0)

                # ---- z_a, z_v ----
                for li, nm in enumerate(("a", "v")):
                    pz = pb.tile([128, 4, B], F32, tag="pz")
                    for m in range(4):   # i, f, o, g
                        nc.tensor.matmul(pz[:, m, :], W(f"z_{nm}", m, 0), xa[:, li, :],
                                         start=True, stop=False)
                        nc.tensor.matmul(pz[:, m, :], W(f"z_{nm}", m, 1),
                                         h_cur[:, 2+li, :], start=False, stop=True)
                    for m in range(3):
                        nc.scalar.activation(sig[:, 6+3*li+m, :], pz[:, m, :], AF.Tanh,
                                             bias=BIAS(f"z{nm}_ifo", m), scale=0.5)
                    nc.scalar.activation(tg[:, 2+li, :], pz[:, 3, :], AF.Tanh,
                                         bias=BIAS(f"z{nm}_g", 0), scale=1.0)

                # sigmoid affine: s = 0.5*t + 0.5 (batched, in-place)
                nc.vector.tensor_scalar(sig[:], sig[:], 0.5, 0.5,
                                        mybir.AluOpType.mult, mybir.AluOpType.add)

                # ---- cell update: c_new = sf*c_prev + si*tg ----
                # sig layout: [il0,il1,fl0,fl1, ol0,ol1, ia,fa,oa, iv,fv,ov]
                SI = [0, 1, 6, 9]
                SF = [2, 3, 7, 10]
                SO = [4, 5, 8, 11]
                c_new = cp.tile([128, 4, B], F32, tag="c")
                for m in range(4):
                    tmp = tmpp.tile([128, B], F32)
                    nc.vector.tensor_mul(c_new[:, m, :], sig[:, SF[m], :], c_prev[:, m, :])
                    nc.vector.tensor_mul(tmp, sig[:, SI[m], :], tg[:, m, :])
                    nc.vector.tensor_add(c_new[:, m, :], c_new[:, m, :], tmp)
                cb_new = cbp.tile([128, 4, B], BF16, tag="cb")
                nc.vector.tensor_copy(cb_new, c_new)
                th = thp.tile([128, 4, B], BF16)
                nc.scalar.activation(th[:], c_new[:], AF.Tanh)
                h_new = hp.tile([128, 4, B], BF16, tag="h")
                for m in range(4):
                    nc.vector.tensor_mul(h_new[:, m, :], sig[:, SO[m], :], th[:, m, :])

                # ---- att1 hidden: h1 = relu(w1 @ cStar + b1) ----
                def cstar(k):
                    return cb_prev[:, k, :] if k < 4 else cb_new[:, k-4, :]

                p1 = pb.tile([128, 4, B], F32, tag="pz")
                for m in range(4):
                    for k in range(8):
                        nc.tensor.matmul(p1[:, m, :], W("att1_l1", m, k), cstar(k),
                                         start=(k == 0), stop=(k == 7))
                h1 = h1p.tile([128, 4, B], BF16, tag="h1")
                for m in range(4):
                    nc.scalar.activation(h1[:, m, :], p1[:, m, :], AF.Relu,
                                         bias=BIAS("att1_h", m))

                # ---- att1 out + softmax (exp) ----
                e = ep.tile([128, 8, B], BF16)
                for half in range(2):
                    pz = pb.tile([128, 4, B], F32, tag="pz")
                    for mi in range(4):
                        m = half * 4 + mi
                        for k in range(4):
                            nc.tensor.matmul(pz[:, mi, :], W("att1_l2", m, k), h1[:, k, :],
                                             start=(k == 0), stop=(k == 3))
                    for mi in range(4):
                        nc.scalar.activation(e[:, half*4+mi, :], pz[:, mi, :], AF.Exp,
                                             bias=BIAS("att1_z", half*4+mi))

                # ---- softmax denom + normalize + attend ----
                psm = pb.tile([128, 4, B], F32, tag="pz")
                for k in range(8):
                    nc.tensor.matmul(psm[0:1, 0, :], ones_c[:], e[:, k, :],
                                     start=(k == 0), stop=(k == 7))
                rvec = smp.tile([1, B], mybir.dt.float32r, tag="sv")
                with nc.allow_low_precision(reason="softmax 1/D scale, f32r"):
                    nc.vector.reciprocal(rvec, psm[0:1, 0, :])
                nc.tensor.matmul(psm[:, 1, :], ones_r[:], rvec[:], start=True, stop=True)
                rb = smp.tile([128, B], F32)
                nc.vector.tensor_copy(rb, psm[:, 1, :])
                for k in range(8):
                    nc.vector.tensor_mul(e[:, k, :], e[:, k, :], rb[:])
                for k in range(8):
                    nc.vector.tensor_mul(e[:, k, :], e[:, k, :], cstar(k))

                # ---- att2: cHat = tanh(mlp(attended)) ----
                p2 = pb.tile([128, 4, B], F32, tag="pz")
                for m in range(4):
                    for k in range(8):
                        nc.tensor.matmul(p2[:, m, :], W("att2_l1", m, k), e[:, k, :],
                                         start=(k == 0), stop=(k == 7))
                h2 = h1p.tile([128, 4, B], BF16, tag="h1")
                for m in range(4):
                    nc.scalar.activation(h2[:, m, :], p2[:, m, :], AF.Relu,
                                         bias=BIAS("att2_h", m))
                p3 = pb.tile([128, 4, B], F32, tag="pz")
                for m in range(4):
                    for k in range(4):
                        nc.tensor.matmul(p3[:, m, :], W("att2_l2", m, k), h2[:, k, :],
                                         start=(k == 0), stop=(k == 3))
                chat = chp.tile([128, 4, B], BF16)
                for m in range(4):
                    nc.scalar.activation(chat[:, m, :], p3[:, m, :], AF.Tanh,
                                         bias=BIAS("att2_o", m))

                # ---- g1, g2: sigmoid(mlp(both)) ----
                gbuf = gp.tile([128, 8, B], BF16)
                for gi, gname in enumerate(("g1", "g2")):
                    ph = pb.tile([128, 4, B], F32, tag="pz")
                    for m in range(4):
                        for k in range(12):
                            rhs = e[:, k, :] if k < 8 else mb_cur[:, k-8, :]
                            nc.tensor.matmul(ph[:, m, :], W(f"{gname}_l1", m, k), rhs,
                                             start=(k == 0), stop=(k == 11))
                    hg = h1p.tile([128, 4, B], BF16, tag="h1")
                    for m in range(4):
                        nc.scalar.activation(hg[:, m, :], ph[:, m, :], AF.Relu,
                                             bias=BIAS(f"{gname}_h", m))
                    po = pb.tile([128, 4, B], F32, tag="pz")
                    for m in range(4):
                        for k in range(4):
                            nc.tensor.matmul(po[:, m, :], W(f"{gname}_l2", m, k), hg[:, k, :],
                                             start=(k == 0), stop=(k == 3))
                    for m in range(4):
                        nc.scalar.activation(gbuf[:, gi*4+m, :], po[:, m, :], AF.Tanh,
                                             bias=BIAS(f"{gname}_o", m), scale=0.5)
                nc.vector.tensor_scalar(gbuf[:], gbuf[:], 0.5, 0.5,
                                        mybir.AluOpType.mult, mybir.AluOpType.add)

                # ---- mem update ----
                mem_new = mp.tile([128, 4, B], F32, tag="mem")
                for m in range(4):
                    tmp = tmpp.tile([128, B], F32)
                    nc.vector.tensor_mul(mem_new[:, m, :], gbuf[:, m, :], mem_cur[:, m, :])
                    nc.vector.tensor_mul(tmp, gbuf[:, 4+m, :], chat[:, m, :])
                    nc.vector.tensor_add(mem_new[:, m, :], mem_new[:, m, :], tmp)
                mb_new = mbp.tile([128, 4, B], BF16, tag="mb")
                nc.vector.tensor_copy(mb_new, mem_new)

                c_prev = c_new
                cb_prev = cb_new
                h_cur = h_new
                mem_cur = mem_new
                mb_cur = mb_new

            # ---- output MLP (weights streamed into the freed e-pool slot) ----
            wo1 = ep.tile([128, 4096], BF16, tag="e")
            _o1 = _W_OFFS["out_l1"][0][0][0]
            nc.gpsimd.dma_start(wo1, d_wb[:, _o1:_o1+4096])
            pout = pb.tile([128, 4, B], F32, tag="pz")
            for m in range(4):
                for k in range(8):
                    rhs = h_cur[:, k, :] if k < 4 else mb_cur[:, k-4, :]
                    nc.tensor.matmul(pout[:, m, :], WO1(m, k), rhs,
                                     start=(k == 0), stop=(k == 7))
            ho = h1p.tile([128, 4, B], BF16, tag="h1")
            for m in range(4):
                nc.scalar.activation(ho[:, m, :], pout[:, m, :], AF.Relu,
                                     bias=BIAS("out_h", m))
            pf = pb.tile([128, 4, B], F32, tag="pz")
            for k in range(4):
                nc.tensor.matmul(pf[0:1, 0, :], wo2[:, k:k+1], ho[:, k, :],
                                 start=(k == 0), stop=(k == 3))
            ysb = smp.tile([1, B], F32, tag="sv")
            nc.scalar.activation(ysb, pf[0:1, 0, :], AF.Identity,
                                 bias=bb[0:1, _B_OFFS["out_o"]:_B_OFFS["out_o"]+1])
            nc.gpsimd.dma_start(d_y, ysb)

    nc.compile()
    _NC_CACHE[key] = nc
    return nc


# ---------------- entry point ----------------
def prepare_in_maps(x_idx, x_av, c_l, c_a, c_v, mem, embed, params):
    x_idx = np.asarray(x_idx)
    x_av = np.asarray(x_av, np.float32)
    embed = np.asarray(embed, np.float32)
    params = {k: np.asarray(v, np.float32) for k, v in params.items()}

    wb = _pack_weights(params)
    bb = _pack_biases(params)
    embp = np.zeros((VOCAB, EP), np.float32)
    embp[:, :D_EMB] = embed
    embp = embp.astype(ml_dtypes.bfloat16)

    in_maps = []
    for c in range(NCORES):
        n0 = c * B
        # indices: [128, T*32] int16 (out col j <- idx[j%16, j//16], tiled x8)
        cols = []
        for t in range(T):
            cols.append(np.asarray(x_idx[t, n0:n0+B], np.int16).reshape(32, 16).T)
        idx = np.tile(np.concatenate(cols, axis=1), (8, 1))

        xav = np.ascontiguousarray(
            x_av[:T, n0:n0+B, :].transpose(0, 2, 1)       # [T, 256, B]
            .reshape(T, 2, 128, B).transpose(0, 2, 1, 3)  # [T, 128, 2, B]
        ).astype(ml_dtypes.bfloat16)

        cl = _feat_major(np.asarray(c_l, np.float32), n0)   # [128, 2, B]
        ca = _feat_major(np.asarray(c_a, np.float32), n0)   # [128, 1, B]
        cv = _feat_major(np.asarray(c_v, np.float32), n0)
        c0 = np.concatenate([cl, ca, cv], axis=1).astype(np.float32)
        m0 = _feat_major(np.asarray(mem, np.float32), n0).astype(np.float32)

        in_maps.append({
            "wb": wb, "bb": bb, "emb": embp, "idx": idx,
            "xav": xav, "c0": c0, "m0": m0,
        })
    return in_maps


def kernel(x_idx, x_av, c_l, c_a, c_v, mem, embed, params):
    in_maps = prepare_in_maps(x_idx, x_av, c_l, c_a, c_v, mem, embed, params)
    nc = _build()
    kwargs = {}
    if os.environ.get("MFN_TRACE"):
        kwargs = dict(trace=True, tmpdir=os.environ.get("MFN_TRACE_DIR") or None)
    res = run_bass_kernel_spmd(nc, in_maps, core_ids=list(range(NCORES)), **kwargs)
    global LAST_RESULTS
    LAST_RESULTS = res
    out = np.concatenate([res.results[c]["y"][0] for c in range(NCORES)])
    return out.reshape(NFULL, 1).astype(np.float32)


LAST_RESULTS = None
